# revision 5
# baseline (speedup 1.0000x reference)
"""Multi-head attention (B=4, L=2048, D=768, H=12) on 8 TRN2 NeuronCores.

Sharding: core c handles batch b=c//2, head-group g=c%2 (6 heads each).
Each core computes the qkv projection for its heads, flash-style attention
(scores never leave SBUF/PSUM), and its partial output projection.
Host sums the two partial projections per batch element and adds b_out.

Heads are processed in pairs (2p, 2p+1). Q/K live in a [128, slot, L]
layout with head 2p on partitions 0-63 and head 2p+1 on partitions
64-127, so the two heads' score matmuls (K=64 contraction) run
concurrently on the PE array as row-tiled 64x128 tiles (tile_position
(0,0) / (64,0), auto-derived from the operand partition bases). The
attention*V matmuls keep the ones-column trick (M=65) so the softmax
denominator rides along in the same instruction.

The kernel is ACT-bound: 192 exp instructions of [128,1024] dominate.
Projection matmuls are interleaved as fillers in the PE stream sized to
the per-iteration ACT slack; the softmax normalize multiplies straight
out of PSUM (no intermediate copy).
"""

import sys

sys.path.insert(0, "/opt/trn_rl_repo")

import numpy as np

B, L, D = 4, 2048, 768
H, DH = 12, 64
HPC = 6  # heads per core
NP = 3  # head pairs per core
N_CORES = 8
QK = 2 * HPC * DH  # 768 qk-projection rows per core
V_W = HPC * (DH + 1)  # v tile width: 6 heads x (64 dims + ones col)

_state = None


def _emit(nc, tc, tile, mybir, bass, nrep=1):
    f32 = mybir.dt.float32
    bf16 = mybir.dt.bfloat16
    Exp = mybir.ActivationFunctionType.Exp

    xT = nc.declare_dram_parameter("xT", [D, L], bf16, isOutput=False)
    w_qk = nc.declare_dram_parameter("w_qk", [D, QK], bf16, isOutput=False)
    b_qk = nc.declare_dram_parameter("b_qk", [128, QK // 128], f32, isOutput=False)
    w_v = nc.declare_dram_parameter("w_v", [D, HPC * DH], bf16, isOutput=False)
    b_v = nc.declare_dram_parameter("b_v", [1, HPC * DH], bf16, isOutput=False)
    w_out = nc.declare_dram_parameter("w_out", [HPC * DH, D], bf16, isOutput=False)
    outT = nc.declare_dram_parameter("outT", [D, L], f32, isOutput=True)

    KT = D // 128  # 6 contraction tiles for the qkv projection
    NVC = HPC * DH  # 384 v columns
    LT = L // 128  # 16 seq tiles
    HL = L // 2  # 1024-query half for the attention inner loop

    from contextlib import ExitStack, nullcontext

    with tc.For_i(0, nrep, 1) if nrep > 1 else nullcontext(), ExitStack() as ctx:
        persist = ctx.enter_context(tc.tile_pool(name="persist", bufs=1))
        # slot p = Q of pair p (head 2p on parts 0-63, 2p+1 on 64-127);
        # slot 3+p = K of pair p, same split.
        qkt = persist.tile([128, 2 * NP, L], bf16, tag="qkt")
        v = persist.tile([128, LT, V_W], bf16, tag="v")
        at = persist.tile([128, NP, L], bf16, tag="at")
        wout_s = persist.tile([128, HPC * DH // 128, D], bf16, tag="wout")
        bqk_s = persist.tile([128, QK // 128], f32, tag="bqk")
        ones = persist.tile([1, 128], bf16, tag="ones")

        xt = persist.tile([128, KT, L], bf16, tag="xt")
        wqk_s = persist.tile([128, KT, QK], bf16, tag="wqk")
        wv_s = persist.tile([128, KT, NVC], bf16, tag="wv")
        bv_s = persist.tile([1, NVC], bf16, tag="bv")

        for k in range(KT):
            nc.sync.dma_start(out=xt[:, k, :], in_=xT[k * 128 : (k + 1) * 128, :])
            nc.gpsimd.dma_start(out=wv_s[:, k, :], in_=w_v[k * 128 : (k + 1) * 128, :])
            nc.gpsimd.dma_start(
                out=wqk_s[:, k, :], in_=w_qk[k * 128 : (k + 1) * 128, :]
            )
        nc.sync.dma_start(out=bv_s, in_=b_v[:, :])
        nc.sync.dma_start(out=bqk_s, in_=b_qk[:, :])
        nc.sync.dma_start(out=wout_s, in_=w_out.rearrange("(t p) d -> p t d", p=128))
        nc.vector.memset(ones, 1.0)
        v_heads = v.rearrange("p t (h c) -> p t h c", h=HPC)
        nc.vector.memset(v_heads[:, :, :, DH : DH + 1], 1.0)

        sp = ctx.enter_context(tc.tile_pool(name="sp", bufs=2, space="PSUM"))
        op = ctx.enter_context(tc.tile_pool(name="op", bufs=2, space="PSUM"))
        ep = ctx.enter_context(tc.tile_pool(name="ep", bufs=4))
        rp = ctx.enter_context(tc.tile_pool(name="rp", bufs=2))
        rbp = ctx.enter_context(tc.tile_pool(name="rbp", bufs=2))
        ostage = ctx.enter_context(tc.tile_pool(name="ostage", bufs=2))

        def v_proj_tile(mt):
            ss_t = sp.tile([128, 1024], f32, tag="ss")
            psv = ss_t[:, :NVC]
            for k in range(KT):
                nc.tensor.matmul(
                    psv,
                    lhsT=xt[:, k, mt * 128 : (mt + 1) * 128],
                    rhs=wv_s[:, k, :],
                    start=(k == 0),
                    stop=False,
                )
            nc.tensor.matmul(
                psv, lhsT=ones[0:1, :], rhs=bv_s[0:1, :], start=False, stop=True
            )
            nc.vector.tensor_copy(
                out=v_heads[:, mt, :, 0:DH],
                in_=psv.rearrange("p (h c) -> p h c", c=DH),
            )

        def qk_proj_chunk(m, c):
            # one 128-row m-tile = both heads of a pair (q or k)
            ss_t = sp.tile([128, 1024], f32, tag="ss")
            ps = ss_t[:, :512]
            for k in range(KT):
                nc.tensor.matmul(
                    ps,
                    lhsT=wqk_s[:, k, m * 128 : (m + 1) * 128],
                    rhs=xt[:, k, c * 512 : (c + 1) * 512],
                    start=(k == 0),
                    stop=(k == KT - 1),
                )
            nc.vector.tensor_scalar_add(
                out=qkt[:, m, c * 512 : (c + 1) * 512],
                in0=ps,
                scalar1=bqk_s[:, m : m + 1],
            )

        def attn_pair(p, fillers=()):
            fillers = list(fillers)
            qt = qkt[:, p, :]
            kt = qkt[:, NP + p, :]
            for lqh in range(2):
                qsl = slice(lqh * HL, (lqh + 1) * HL)
                po = [op.tile([65, HL], f32, tag="po", name=f"po{i}") for i in range(2)]
                av_prev = None
                for mk in range(LT):
                    if fillers:
                        fillers.pop(0)()
                    ss = [sp.tile([128, 1024], f32, tag="ss", name=f"ss{i}") for i in range(2)]
                    # row-tiled score matmuls: both heads concurrently
                    for j in range(2):
                        for hh in range(2):
                            hs = slice(64 * hh, 64 * hh + 64)
                            nc.tensor.matmul(
                                ss[hh][:, j * 512 : (j + 1) * 512],
                                lhsT=kt[hs, mk * 128 : (mk + 1) * 128],
                                rhs=qt[hs, lqh * HL + j * 512 : lqh * HL + (j + 1) * 512],
                                start=True,
                                stop=True,
                            )
                    ex = [ep.tile([128, 1024], bf16, tag="ex", name=f"ex{i}") for i in range(2)]
                    for hh in range(2):
                        nc.scalar.activation(
                            out=ex[hh], in_=ss[hh], func=Exp, scale=0.125
                        )
                    if av_prev is not None:
                        av_prev()

                    def av_now(mk=mk, ex=ex):
                        for hh in range(2):
                            h = 2 * p + hh
                            for j in range(2):
                                nc.tensor.matmul(
                                    po[hh][:, j * 512 : (j + 1) * 512],
                                    lhsT=v[:, mk, h * 65 : (h + 1) * 65],
                                    rhs=ex[hh][:, j * 512 : (j + 1) * 512],
                                    start=(mk == 0),
                                    stop=(mk == LT - 1),
                                )

                    av_prev = av_now
                av_prev()
                for hh in range(2):
                    rsh = rp.tile([1, HL], f32, tag="rsh")
                    nc.vector.reciprocal(out=rsh, in_=po[hh][64:65, :])
                    rb = rbp.tile([128, HL], f32, tag="rb")
                    nc.gpsimd.partition_broadcast(rb, rsh[0:1, :], channels=128)
                    nc.vector.tensor_mul(
                        out=at[64 * hh : 64 * hh + 64, p, qsl],
                        in0=po[hh][0:64, :],
                        in1=rb[0:64, :],
                    )
            while fillers:
                fillers.pop(0)()

        def qkf(m, c):
            return lambda: qk_proj_chunk(m, c)

        def vf(mt):
            return lambda: v_proj_tile(mt)

        # prelude: just enough for pair 0 / lqh 0 to start
        for c in range(2):
            qk_proj_chunk(0, c)
        qk_proj_chunk(3, 0)
        for mt in range(4):
            v_proj_tile(mt)
        # fillers ride in the attention mk loops; deadlines (perf, not
        # correctness — Tile deps guarantee order): qk(3+p, c) before pair p
        # reaches mk=4c; v(t) before av hits mk=t; qk(p, 2/3) before lqh1.
        f0 = [vf(4), qkf(3, 1), vf(5), vf(6), vf(7), qkf(3, 2), vf(8), vf(9),
              vf(10), vf(11), qkf(3, 3), vf(12), vf(13), vf(14), vf(15),
              qkf(0, 2),
              # lqh1 slots
              qkf(0, 3), qkf(1, 0), qkf(1, 1), qkf(4, 0), qkf(4, 1),
              qkf(4, 2), qkf(4, 3), qkf(1, 2)]
        f1 = [qkf(1, 3), qkf(2, 0), qkf(2, 1), qkf(5, 0), qkf(5, 1),
              qkf(5, 2), qkf(5, 3),
              # lqh1 slots
              qkf(2, 2), qkf(2, 3)]
        attn_pair(0, f0)
        attn_pair(1, f1)
        attn_pair(2)

        # output projection: psum -> sbuf staging -> dram
        for m in range(D // 128):
            for c in range(4):
                ss_t = sp.tile([128, 1024], f32, tag="ss")
                pso = ss_t[:, :512]
                for k in range(HPC * DH // 128):
                    nc.tensor.matmul(
                        pso,
                        lhsT=wout_s[:, k, m * 128 : (m + 1) * 128],
                        rhs=at[:, k, c * 512 : (c + 1) * 512],
                        start=(k == 0),
                        stop=(k == HPC * DH // 128 - 1),
                    )
                ot = ostage.tile([128, 512], f32, tag="ot")
                nc.vector.tensor_copy(out=ot, in_=pso)
                nc.sync.dma_start(
                    out=outT[m * 128 : (m + 1) * 128, c * 512 : (c + 1) * 512],
                    in_=ot,
                )


def _build(nrep=1):
    global _state
    if nrep == 1 and _state is not None:
        return _state
    import concourse.bacc as bacc
    import concourse.tile as tile
    import concourse.bass as bass
    from concourse import mybir

    nc = bacc.Bacc("TRN2", target_bir_lowering=False)
    with tile.TileContext(nc) as tc:
        _emit(nc, tc, tile, mybir, bass, nrep=nrep)
    nc.compile()
    if nrep != 1:
        return nc
    _state = nc
    return nc


def make_in_maps(x, W_qkv, b_qkv, W_out):
    """Host-side sharding: per-core input dict."""
    import ml_dtypes

    bf = ml_dtypes.bfloat16
    x = np.asarray(x, np.float32).astype(bf)
    W_qkv = np.asarray(W_qkv, np.float32).astype(bf)
    b_qkv = np.asarray(b_qkv, np.float32)
    W_out = np.asarray(W_out, np.float32).astype(bf)
    in_maps = []
    for c in range(N_CORES):
        b, g = divmod(c, 2)
        qs = slice(384 * g, 384 * g + 384)
        ks = slice(768 + 384 * g, 768 + 384 * g + 384)
        vs = slice(1536 + 384 * g, 1536 + 384 * g + 384)
        bqk = np.concatenate([b_qkv[qs], b_qkv[ks]])
        in_maps.append(
            {
                "xT": np.ascontiguousarray(x[b].T),
                "w_qk": np.ascontiguousarray(
                    np.concatenate([W_qkv[:, qs], W_qkv[:, ks]], axis=1)
                ),
                "b_qk": np.ascontiguousarray(bqk.reshape(QK // 128, 128).T),
                "w_v": np.ascontiguousarray(W_qkv[:, vs]),
                "b_v": np.ascontiguousarray(b_qkv[vs][None, :].astype(bf)),
                "w_out": np.ascontiguousarray(W_out[384 * g : 384 * g + 384, :]),
            }
        )
    return in_maps


def gather(results, b_out):
    """Host-side unshard: sum the two partial projections per batch + bias."""
    b_out = np.asarray(b_out, np.float32)
    out = np.empty((B, L, D), np.float32)
    for b in range(B):
        yt = results[2 * b]["outT"] + results[2 * b + 1]["outT"]
        out[b] = yt.T + b_out
    return out


def kernel(x, W_qkv, b_qkv, W_out, b_out):
    from concourse.bass_utils import run_bass_kernel_spmd

    nc = _build()
    in_maps = make_in_maps(x, W_qkv, b_qkv, W_out)
    res = run_bass_kernel_spmd(nc, in_maps, list(range(N_CORES)))
    return gather(res.results, b_out)


# revision 9
# speedup vs baseline: 1.8263x; 1.8263x over previous
"""Multi-head attention (B=4, L=2048, D=768, H=12) on 8 TRN2 NeuronCores.

Sharding: core c handles batch b=c//2, head-group g=c%2 (6 heads each).
Each core computes the qkv projection for its heads, flash-style attention
(scores never leave SBUF/PSUM), and its partial output projection.
Host sums the two partial projections per batch element and adds b_out.

Every matmul stream alternates PE tile positions so LDWEIGHTS/drain of
consecutive instructions never contend for the same array row/col groups
(same-position chains measure ~460ns per N=512 matmul on HW; alternating
streams hit the bf16 roofline, ~50ns per K=64 half):
  - scores: heads (2p, 2p+1) packed on partitions 0-63 / 64-127 of a
    [128, slot, L] Q/K layout -> row-tiled (0,0)/(64,0) pairs.
  - attn*V: the pair's V matmuls col-tiled (0,0)/(0,64) into the top and
    bottom halves of one [128, 1024] PSUM accumulator.
  - softmax denominators: ones-weight M=1 matmuls col-tiled (0,0)/(0,32)
    into a [33, 1024] PSUM accumulator.
  - projections: each K=128 k-tile split into K=64 row-halves alternating
    (0,0)/(64,0) into two separate PSUM banks, recombined by DVE.

The kernel is ACT-bound: 192 exp instructions of [128,1024] dominate;
projection work rides as fillers inside the attention mk loop.
"""

import sys

sys.path.insert(0, "/opt/trn_rl_repo")

import numpy as np

B, L, D = 4, 2048, 768
H, DH = 12, 64
HPC = 6  # heads per core
NP = 3  # head pairs per core
N_CORES = 8
QK = 2 * HPC * DH  # 768 qk-projection rows per core

_state = None


def _emit(nc, tc, tile, mybir, bass, nrep=1, debug=False):
    f32 = mybir.dt.float32
    bf16 = mybir.dt.bfloat16
    Exp = mybir.ActivationFunctionType.Exp

    xT = nc.declare_dram_parameter("xT", [D, L], bf16, isOutput=False)
    w_qk = nc.declare_dram_parameter("w_qk", [D, QK], bf16, isOutput=False)
    b_qk = nc.declare_dram_parameter("b_qk", [128, QK // 128], f32, isOutput=False)
    w_v = nc.declare_dram_parameter("w_v", [D, HPC * DH], bf16, isOutput=False)
    b_v = nc.declare_dram_parameter("b_v", [1, HPC * DH], bf16, isOutput=False)
    w_out = nc.declare_dram_parameter("w_out", [HPC * DH, D], bf16, isOutput=False)
    outT = nc.declare_dram_parameter("outT", [D, L], f32, isOutput=True)
    if debug:
        qkt_d = nc.declare_dram_parameter("qkt_d", [128, 6 * L], bf16, isOutput=True)
        v_d = nc.declare_dram_parameter("v_d", [128, 16 * HPC * DH], bf16, isOutput=True)
        at_d = nc.declare_dram_parameter("at_d", [128, NP * L], bf16, isOutput=True)
        dn_d = nc.declare_dram_parameter("dn_d", [33, 12 * 1024], f32, isOutput=True)
        po_d = nc.declare_dram_parameter("po_d", [128, 12 * 1024], f32, isOutput=True)

    KT = D // 128  # 6 contraction tiles for the qkv projection
    NVC = HPC * DH  # 384 v columns
    LT = L // 128  # 16 seq tiles
    HL = L // 2  # 1024-query half for the attention inner loop

    from contextlib import ExitStack, nullcontext

    with tc.For_i(0, nrep, 1) if nrep > 1 else nullcontext(), ExitStack() as ctx:
        persist = ctx.enter_context(tc.tile_pool(name="persist", bufs=1))
        # slot p = Q of pair p (head 2p on parts 0-63, 2p+1 on 64-127);
        # slot 3+p = K of pair p, same split.
        qkt = persist.tile([128, 2 * NP, L], bf16, tag="qkt")
        v = persist.tile([128, LT, NVC], bf16, tag="v")
        at = persist.tile([128, NP, L], bf16, tag="at")
        wout_s = persist.tile([128, HPC * DH // 128, D], bf16, tag="wout")
        bqk_s = persist.tile([128, QK // 128], f32, tag="bqk")
        ones = persist.tile([1, 128], bf16, tag="ones")
        ones_col = persist.tile([128, 1], bf16, tag="ones_col")

        xt = persist.tile([128, KT, L], bf16, tag="xt")
        wqk_s = persist.tile([128, KT, QK], bf16, tag="wqk")
        wv_s = persist.tile([128, KT, NVC], bf16, tag="wv")
        bv_s = persist.tile([1, NVC], bf16, tag="bv")

        for k in range(KT):
            nc.sync.dma_start(out=xt[:, k, :], in_=xT[k * 128 : (k + 1) * 128, :])
            nc.gpsimd.dma_start(out=wv_s[:, k, :], in_=w_v[k * 128 : (k + 1) * 128, :])
            nc.gpsimd.dma_start(
                out=wqk_s[:, k, :], in_=w_qk[k * 128 : (k + 1) * 128, :]
            )
        nc.sync.dma_start(out=bv_s, in_=b_v[:, :])
        nc.sync.dma_start(out=bqk_s, in_=b_qk[:, :])
        nc.sync.dma_start(out=wout_s, in_=w_out.rearrange("(t p) d -> p t d", p=128))
        nc.vector.memset(ones, 1.0)
        nc.vector.memset(ones_col, 1.0)

        sp = ctx.enter_context(tc.tile_pool(name="sp", bufs=2, space="PSUM"))
        op = ctx.enter_context(tc.tile_pool(name="op", bufs=1, space="PSUM"))
        dp = ctx.enter_context(tc.tile_pool(name="dp", bufs=1, space="PSUM"))
        ep = ctx.enter_context(tc.tile_pool(name="ep", bufs=6))
        rp = ctx.enter_context(tc.tile_pool(name="rp", bufs=2))
        rbp = ctx.enter_context(tc.tile_pool(name="rbp", bufs=2))
        tp = ctx.enter_context(tc.tile_pool(name="tp", bufs=2))
        ostage = ctx.enter_context(tc.tile_pool(name="ostage", bufs=2))

        def halved_accum(psT, psB, lhs_of_k, rhs_of_k, nk):
            """K=128 k-tiles split into alternating K=64 row-halves into two
            separate banks; caller recombines psT+psB."""
            for k in range(nk):
                lh, rh = lhs_of_k(k), rhs_of_k(k)
                nc.tensor.matmul(
                    psT, lhsT=lh[0:64], rhs=rh[0:64],
                    start=(k == 0), stop=(k == nk - 1),
                )
                nc.tensor.matmul(
                    psB, lhsT=lh[64:128], rhs=rh[64:128],
                    start=(k == 0), stop=(k == nk - 1),
                )

        def v_proj_tile(mt):
            ss_t = sp.tile([128, 1024], f32, tag="ss")
            psT, psB = ss_t[:, 0:NVC], ss_t[:, 512 : 512 + NVC]
            halved_accum(
                psT, psB,
                lambda k: xt[:, k, mt * 128 : (mt + 1) * 128],
                lambda k: wv_s[:, k, :],
                KT,
            )
            # bias rides on psB via a rank-1 matmul (K=1)
            nc.tensor.matmul(
                psB, lhsT=ones[0:1, :], rhs=bv_s[0:1, :], start=False, stop=True
            )
            vtmp = tp.tile([128, NVC], bf16, tag="vtmp")
            nc.vector.tensor_copy(out=vtmp, in_=psT)
            nc.vector.tensor_add(out=v[:, mt, :], in0=psB, in1=vtmp)

        def qk_proj_chunk(m, c):
            # one 128-row m-tile = both heads of a pair (q or k)
            ss_t = sp.tile([128, 1024], f32, tag="ss")
            psT, psB = ss_t[:, 0:512], ss_t[:, 512:1024]
            halved_accum(
                psT, psB,
                lambda k: wqk_s[:, k, m * 128 : (m + 1) * 128],
                lambda k: xt[:, k, c * 512 : (c + 1) * 512],
                KT,
            )
            qtmp = tp.tile([128, 512], bf16, tag="qtmp")
            nc.vector.tensor_scalar_add(
                out=qtmp, in0=psT, scalar1=bqk_s[:, m : m + 1]
            )
            nc.vector.tensor_add(
                out=qkt[:, m, c * 512 : (c + 1) * 512], in0=psB, in1=qtmp
            )

        def attn_pair(p, fillers=()):
            fillers = list(fillers)
            qt = qkt[:, p, :]
            kt = qkt[:, NP + p, :]
            for lqh in range(2):
                qsl = slice(lqh * HL, (lqh + 1) * HL)
                po = op.tile([128, HL], f32, tag="po")
                dn = dp.tile([33, HL], f32, tag="dn")
                exs = []

                def av_one(mk, hh):
                    # col-tiled AV: head A -> rows 0-63 (0,0), B -> 64-127
                    # (0,64), same bank; start/stop are footprint-scoped.
                    ex = exs[mk][hh]
                    for j in range(2):
                        jm = slice(j * 512, (j + 1) * 512)
                        yield lambda j=j, jm=jm: nc.tensor.matmul(
                            po[64 * hh : 64 * hh + 64, jm],
                            lhsT=v[:, mk, (2 * p + hh) * DH : (2 * p + hh + 1) * DH],
                            rhs=ex[:, jm],
                            start=(mk == 0),
                            stop=(mk == LT - 1),
                            skip_group_check=True,
                        )

                def den_one(mk, hh):
                    ex = exs[mk][hh]
                    for j in range(2):
                        jm = slice(j * 512, (j + 1) * 512)
                        yield lambda j=j, jm=jm: nc.tensor.matmul(
                            dn[32 * hh : 32 * hh + 1, jm],
                            lhsT=ones_col[:, 0:1],
                            rhs=ex[:, jm],
                            start=(mk == 0),
                            stop=(mk == LT - 1),
                            skip_group_check=True,
                        )

                def avden_block(mka, mkb):
                    """Interleave A-head work for mka with B-head work for mkb
                    so consecutive matmuls alternate PE col groups."""
                    streams = []
                    if mka is not None:
                        streams.append(list(av_one(mka, 0)) + list(den_one(mka, 0)))
                    if mkb is not None:
                        streams.append(list(av_one(mkb, 1)) + list(den_one(mkb, 1)))
                    n = max((len(s) for s in streams), default=0)
                    for i in range(n):
                        for s in streams:
                            if i < len(s):
                                s[i]()

                def scores(mk, hh, ss):
                    for j in range(2):
                        hs = slice(64 * hh, 64 * hh + 64)
                        nc.tensor.matmul(
                            ss[:, j * 512 : (j + 1) * 512],
                            lhsT=kt[hs, mk * 128 : (mk + 1) * 128],
                            rhs=qt[hs, lqh * HL + j * 512 : lqh * HL + (j + 1) * 512],
                            start=True,
                            stop=True,
                        )

                for mk in range(LT):
                    ssA = sp.tile([128, 1024], f32, tag="ss")
                    scores(mk, 0, ssA)
                    exA = ep.tile([128, 1024], bf16, tag="ex")
                    nc.scalar.activation(out=exA, in_=ssA, func=Exp, scale=0.125)
                    avden_block(mk - 1 if mk >= 1 else None,
                                mk - 2 if mk >= 2 else None)
                    ssB = sp.tile([128, 1024], f32, tag="ss")
                    scores(mk, 1, ssB)
                    exB = ep.tile([128, 1024], bf16, tag="ex")
                    nc.scalar.activation(out=exB, in_=ssB, func=Exp, scale=0.125)
                    exs.append((exA, exB))
                    if fillers:
                        fillers.pop(0)()
                        if fillers:
                            fillers.pop(0)()
                # drain the lagged AV/den work
                avden_block(LT - 1, LT - 2)
                avden_block(None, LT - 1)
                if debug:
                    dnst = tp.tile([33, HL], f32, tag="dnst")
                    nc.vector.tensor_copy(out=dnst, in_=dn)
                    nc.sync.dma_start(
                        out=dn_d[:, (4 * p + 2 * lqh) * 512 : (4 * p + 2 * lqh + 2) * 512],
                        in_=dnst,
                    )
                    post = tp.tile([128, HL], f32, tag="post")
                    nc.vector.tensor_copy(out=post, in_=po)
                    nc.sync.dma_start(
                        out=po_d[:, (4 * p + 2 * lqh) * 512 : (4 * p + 2 * lqh + 2) * 512],
                        in_=post,
                    )
                for hh in range(2):
                    rsh = rp.tile([1, HL], f32, tag="rsh")
                    nc.vector.reciprocal(out=rsh, in_=dn[32 * hh : 32 * hh + 1, :])
                    rb = rbp.tile([128, HL], f32, tag="rb")
                    nc.gpsimd.partition_broadcast(rb, rsh[0:1, :], channels=128)
                    nc.vector.tensor_mul(
                        out=at[64 * hh : 64 * hh + 64, p, qsl],
                        in0=po[64 * hh : 64 * hh + 64, :],
                        in1=rb[0:64, :],
                    )
            while fillers:
                fillers.pop(0)()

        def qkf(m, c):
            return lambda: qk_proj_chunk(m, c)

        def vf(mt):
            return lambda: v_proj_tile(mt)

        # prelude: just enough for pair 0 / lqh 0 to start
        for c in range(2):
            qk_proj_chunk(0, c)
        qk_proj_chunk(3, 0)
        qk_proj_chunk(3, 1)
        for mt in range(4):
            v_proj_tile(mt)
        # fillers ride in the attention mk loops; deadlines (perf, not
        # correctness — Tile deps guarantee order): qk(3+p, c) before pair p
        # reaches mk=4c; v(t) before av hits mk=t; qk(p, 2/3) before lqh1.
        f0 = [vf(4), vf(5), vf(6), vf(7), qkf(3, 2), vf(8), vf(9), vf(10),
              vf(11), qkf(3, 3), vf(12), vf(13), vf(14), vf(15), qkf(0, 2),
              qkf(0, 3),
              # lqh1 slots
              qkf(1, 0), qkf(1, 1), qkf(4, 0), qkf(4, 1), qkf(4, 2),
              qkf(4, 3), qkf(1, 2), qkf(1, 3)]
        f1 = [qkf(2, 0), qkf(2, 1), qkf(5, 0), qkf(5, 1), qkf(5, 2),
              qkf(5, 3), qkf(2, 2), qkf(2, 3)]
        attn_pair(0, f0)
        attn_pair(1, f1)
        attn_pair(2)

        if debug:
            nc.sync.dma_start(out=qkt_d.rearrange("p (m l) -> p m l", m=6), in_=qkt)
            nc.sync.dma_start(out=v_d.rearrange("p (t w) -> p t w", t=16), in_=v)
            nc.sync.dma_start(out=at_d.rearrange("p (m l) -> p m l", m=NP), in_=at)

        # output projection: halved k-tiles -> two banks -> DVE combine -> dram
        for m in range(D // 128):
            for c in range(4):
                ss_t = sp.tile([128, 1024], f32, tag="ss")
                psT, psB = ss_t[:, 0:512], ss_t[:, 512:1024]
                halved_accum(
                    psT, psB,
                    lambda k: wout_s[:, k, m * 128 : (m + 1) * 128],
                    lambda k: at[:, k, c * 512 : (c + 1) * 512],
                    HPC * DH // 128,
                )
                ot1 = ostage.tile([128, 512], f32, tag="ot1")
                nc.vector.tensor_copy(out=ot1, in_=psT)
                ot = ostage.tile([128, 512], f32, tag="ot")
                nc.vector.tensor_add(out=ot, in0=psB, in1=ot1)
                nc.sync.dma_start(
                    out=outT[m * 128 : (m + 1) * 128, c * 512 : (c + 1) * 512],
                    in_=ot,
                )


def _build(nrep=1, debug=False):
    global _state
    if not debug and nrep == 1 and _state is not None:
        return _state
    import concourse.bacc as bacc
    import concourse.tile as tile
    import concourse.bass as bass
    from concourse import mybir

    nc = bacc.Bacc("TRN2", target_bir_lowering=False)
    with tile.TileContext(nc) as tc:
        _emit(nc, tc, tile, mybir, bass, nrep=nrep, debug=debug)
    nc.compile()
    if nrep != 1 or debug:
        return nc
    _state = nc
    return nc


def make_in_maps(x, W_qkv, b_qkv, W_out):
    """Host-side sharding: per-core input dict."""
    import ml_dtypes

    bf = ml_dtypes.bfloat16
    x = np.asarray(x, np.float32).astype(bf)
    W_qkv = np.asarray(W_qkv, np.float32).astype(bf)
    b_qkv = np.asarray(b_qkv, np.float32)
    W_out = np.asarray(W_out, np.float32).astype(bf)
    in_maps = []
    for c in range(N_CORES):
        b, g = divmod(c, 2)
        qs = slice(384 * g, 384 * g + 384)
        ks = slice(768 + 384 * g, 768 + 384 * g + 384)
        vs = slice(1536 + 384 * g, 1536 + 384 * g + 384)
        bqk = np.concatenate([b_qkv[qs], b_qkv[ks]])
        in_maps.append(
            {
                "xT": np.ascontiguousarray(x[b].T),
                "w_qk": np.ascontiguousarray(
                    np.concatenate([W_qkv[:, qs], W_qkv[:, ks]], axis=1)
                ),
                "b_qk": np.ascontiguousarray(bqk.reshape(QK // 128, 128).T),
                "w_v": np.ascontiguousarray(W_qkv[:, vs]),
                "b_v": np.ascontiguousarray(b_qkv[vs][None, :].astype(bf)),
                "w_out": np.ascontiguousarray(W_out[384 * g : 384 * g + 384, :]),
            }
        )
    return in_maps


def gather(results, b_out):
    """Host-side unshard: sum the two partial projections per batch + bias."""
    b_out = np.asarray(b_out, np.float32)
    out = np.empty((B, L, D), np.float32)
    for b in range(B):
        yt = results[2 * b]["outT"] + results[2 * b + 1]["outT"]
        out[b] = yt.T + b_out
    return out


def kernel(x, W_qkv, b_qkv, W_out, b_out):
    from concourse.bass_utils import run_bass_kernel_spmd

    nc = _build()
    in_maps = make_in_maps(x, W_qkv, b_qkv, W_out)
    res = run_bass_kernel_spmd(nc, in_maps, list(range(N_CORES)))
    return gather(res.results, b_out)
